# revision 48
# baseline (speedup 1.0000x reference)
"""Causal multi-head self-attention with RoPE on 8 Trainium2 NeuronCores.

Sharding: batch (4) x head-half (2) -> 8 self-contained cores. Each core
computes Q/K/V projections for its 8 heads, RoPE, causal flash-style
attention (scores kept transposed [key, query] so probs feed the V matmul
with no on-device transpose), and a partial output projection over its 512
context features. The two partial outputs per batch are summed on host
(the "all-reduce after output projection" of the tensor-parallel split).

Device layout notes:
- Q/K/V projection matmuls run in bf16 (x and the three weight matrices
  are host-cast): fp32 HIGH mode streams the PE at half rate, bf16 at
  full rate, so the projections take half the PE time. Attention and the
  output projection stay float32r (full fp32 data, fast PE path).
- Softmax: scores are masked additively via a PE-written -1e30 triangle,
  exponentiated without max-subtraction (scores are bounded; verified),
  and the per-query sums come free from a ones-column appended to V.
- RoPE is evaluated as q*cos + swap(q)*(+-sin) where swap is a DVE
  stream_shuffle partition pair-swap and the sign is folded into the sin
  table.
"""

import sys

sys.path.insert(0, "/opt/trn_rl_repo")

import numpy as np

B, S_FULL, D, H = 4, 2048, 1024, 16
DK = 64  # head dim
HL = 8  # heads per core
DL = HL * DK  # 512 local features
ROPE_THETA = 10000.0
NEG = -1.0e30

_CACHE = {}


def _emit(nc, tc, tensors, S, reps=1):
    import concourse.tile as tile  # noqa: F401
    from concourse import mybir
    from contextlib import ExitStack

    f32, f32r = mybir.dt.float32, mybir.dt.float32r
    bf16 = mybir.dt.bfloat16
    AF = mybir.ActivationFunctionType
    SWAP = [i ^ 1 for i in range(32)]
    NSB = S // 512  # query super-blocks
    NKB = S // 128  # key blocks

    xT, wqT, wkT, wvT, woT = (
        tensors["xT"], tensors["wqT"], tensors["wkT"], tensors["wvT"], tensors["woT"],
    )
    tabs, maskT, ident = tensors["tabs"], tensors["maskT"], tensors["ident"]
    outp = tensors["outp"]

    with ExitStack() as ctx:
        const = ctx.enter_context(tc.tile_pool(name="const", bufs=1))
        wres = ctx.enter_context(tc.tile_pool(name="wres", bufs=1))
        xt_p = ctx.enter_context(tc.tile_pool(name="xt", bufs=10))
        tb_p = ctx.enter_context(tc.tile_pool(name="tb", bufs=2))
        kt_p = ctx.enter_context(tc.tile_pool(name="kt", bufs=1))
        vt_p = ctx.enter_context(tc.tile_pool(name="vt", bufs=1))
        qt_p = ctx.enter_context(tc.tile_pool(name="qt", bufs=1))
        qs_p = ctx.enter_context(tc.tile_pool(name="qs", bufs=2))
        ex_p = ctx.enter_context(tc.tile_pool(name="ex", bufs=4))
        cx_p = ctx.enter_context(tc.tile_pool(name="cx", bufs=1))
        rc_p = ctx.enter_context(tc.tile_pool(name="rc", bufs=1))
        rl_p = ctx.enter_context(tc.tile_pool(name="rl", bufs=1))
        os_p = ctx.enter_context(tc.tile_pool(name="os", bufs=2))
        pp = ctx.enter_context(tc.tile_pool(name="pp", bufs=2, space="PSUM"))
        ps = ctx.enter_context(tc.tile_pool(name="ps", bufs=2, space="PSUM"))
        pc = ctx.enter_context(tc.tile_pool(name="pc", bufs=1, space="PSUM"))

        # resident weights (contraction-major), loaded per 128-row chunk (Pool queue)
        woT_r = woT.rearrange("(t p) o -> p t o", p=128)
        # prefetch x tiles and rope tables for the first super-block before
        # the big weight loads so the first projection starts early
        xts0 = []
        for ct in range(8):
            xt_t = xt_p.tile([128, 512], bf16, tag="xt", name="xt0")
            nc.gpsimd.dma_start(xt_t[:], xT[ct * 128 : (ct + 1) * 128, 0:512])
            xts0.append(xt_t)
        tb_t0 = tb_p.tile([128, 2, 512], f32, tag="tb", name="tb0")
        nc.sync.dma_start(tb_t0[:], tabs[:, :, 0:512])
        wq_t = wres.tile([128, 8, DL], bf16, tag="wq")
        wk_t = wres.tile([128, 8, DL], bf16, tag="wk")
        wv_t = wres.tile([128, 8, DL], bf16, tag="wv")
        wo_t = wres.tile([128, 4, D], f32r, tag="wo")
        for ct in range(8):
            nc.sync.dma_start(wk_t[:, ct, :], wkT[ct * 128 : (ct + 1) * 128, :])
        for w_t, wT in ((wq_t, wqT), (wv_t, wvT)):
            for ct in range(8):
                nc.gpsimd.dma_start(w_t[:, ct, :], wT[ct * 128 : (ct + 1) * 128, :])
        # small constants after the weight loads (needed later than W)
        maskT_t = const.tile([128, 896], bf16, tag="maskT")
        nc.sync.dma_start(maskT_t[:], maskT[:])
        ident_t = const.tile([128, 128], bf16, tag="ident")
        nc.sync.dma_start(ident_t[:], ident[:])
        for it in range(4):
            nc.sync.dma_start(
                wo_t[:, it, :], woT_r[:, it, :].bitcast(f32r)
            )

        # persistent K / V buffers
        kt_tiles = {}
        for hp in range(4):
            for sbk in range(NSB):
                kt_tiles[hp, sbk] = kt_p.tile([128, 512], f32r, tag=f"kt{hp}_{sbk}", name=f"kt{hp}_{sbk}")
        v_tiles = {}
        for kb in range(NKB):
            v_tiles[kb] = vt_p.tile([128, HL, 65], f32r, tag=f"v{kb}", name=f"v{kb}")
            nc.vector.memset(v_tiles[kb][:, :, 64:65].bitcast(f32), 1.0)

        prev_out = [None]
        pending_norm = [None]
        next_x = [None]

        def load_x(s0n, name="xt"):
            xts_n = []
            for ct in range(8):
                xt_t = xt_p.tile([128, 512], bf16, tag="xt", name=name)
                eng = nc.gpsimd if ct % 2 == 0 else nc.sync
                eng.dma_start(xt_t[:], xT[ct * 128 : (ct + 1) * 128, s0n : s0n + 512])
                xts_n.append(xt_t)
            tb_n = tb_p.tile([128, 2, 512], f32, tag="tb", name="tb")
            nc.sync.dma_start(tb_n[:], tabs[:, :, s0n : s0n + 512])
            return xts_n, tb_n

        def emit_norm_a():
            # stage A of a head-pair's softmax normalization: evacuate the
            # denominator row to SBUF reshaped [128,4] (16B/partition DMA) so
            # the iterative-divide reciprocal costs 8 lane-elements, then
            # broadcast the reciprocals to 64 partitions on gpsimd. Emits no
            # DVE op that has to WAIT (the recip is stage B) so it never
            # blocks later DVE work queued behind it.
            hp_, cxs_, cxd_ = pending_norm[0]
            den_s = rc_p.tile([1, 2, 512], f32, tag="dens", name="dens")
            d4 = rc_p.tile([128, 2, 4], f32, tag="d4", name="d4")
            for par in (0, 1):
                nc.vector.tensor_copy(den_s[:, par, :], cxs_[par][64:65, :])
                nc.gpsimd.dma_start(d4[:, par, :], den_s[:, par, :])
            return d4

        def emit_norm_b(d4):
            r4 = rc_p.tile([128, 2, 4], f32, tag="r4", name="r4")
            rc0 = rc_p.tile([1, 512], f32, tag="rc0", name="rc0")
            rc1 = rc_p.tile([1, 512], f32, tag="rc1", name="rc1")
            rb_t = rc_p.tile([64, 2, 512], f32, tag="rb", name="rb")
            nc.vector.reciprocal(r4[:], d4[:])
            for par, rc_t in ((0, rc0), (1, rc1)):
                nc.gpsimd.dma_start(rc_t[:], r4[:, par, :])
                nc.gpsimd.partition_broadcast(rb_t[:, par, :], rc_t[:])
            return rb_t

        def emit_norm_c(rb_t):
            hp_, cxs_, cxd_ = pending_norm[0]
            pending_norm[0] = None
            cxt = cx_p.tile([128, 512], f32r, tag=f"cx{hp_}", name=f"cx{hp_}")
            cxd_[hp_] = cxt
            nc.vector.tensor_mul(cxt[0:64, :], cxs_[0][0:64, :], rb_t[:, 0, :])
            rl_t = rl_p.tile([64, 512], f32r, tag="rl", name="rl")
            nc.vector.tensor_mul(rl_t[:], cxs_[1][0:64, :], rb_t[:, 1, :])
            nc.sync.dma_start(cxt[64:128, :], rl_t[:])

        def emit_norm():
            if pending_norm[0] is None:
                return
            emit_norm_c(emit_norm_b(emit_norm_a()))

        for _rep in range(reps):
         for sb in range(NSB):
            s0 = sb * 512
            if _rep == 0 and sb == 0:
                xts, tb_t = xts0, tb_t0
            else:
                xts, tb_t = load_x(s0)
            # the previous super-block's last norm is flushed in stages woven
            # through this block's projection phase: each stage's ops only
            # execute once their inputs are surely ready, so they never block
            # the DVE/gpsimd queues that feed the rope -> projection pipeline
            norm_d4 = emit_norm_a() if pending_norm[0] is not None else None
            norm_rb = [None]

            # K and Q projections (transposed layout [f, s]) + RoPE. The
            # eight projection groups rotate over FOUR PSUM slots (pp's two
            # plus the attention score pool's two, idle during this phase) so
            # the 3-op DVE rope chain never gates the PE.
            qt_tiles = {}
            gidx = 0
            for w_t, is_k in ((wk_t, True), (wq_t, False)):
                for ft in range(4):
                    if gidx % 4 < 2:
                        pr = pp.tile([128, 512], f32, tag="mm")
                    else:
                        pr = ps.tile([128, 512], f32, tag="sc", name="prs")
                    gidx += 1
                    for ct in range(8):
                        nc.tensor.matmul(
                            pr[:],
                            w_t[:, ct, ft * 128 : (ft + 1) * 128],
                            xts[ct][:],
                            start=(ct == 0),
                            stop=(ct == 7),
                        )
                    # rope: dst = pr * cos + pairswap(pr) * (+-sin); both
                    # readers of pr (mul + shuffle) are issued first so the
                    # PSUM slot frees as fast as PE produces the next group
                    if is_k:
                        dst = kt_tiles[ft, sb]
                    else:
                        dst = qt_p.tile([128, 512], f32r, tag=f"qt{ft}", name=f"qt{ft}")
                        qt_tiles[ft] = dst
                    qs_t = qs_p.tile([128, 512], f32, tag="qs")
                    nc.vector.tensor_mul(dst[:], pr[:], tb_t[:, 0, :])
                    nc.vector.stream_shuffle(qs_t[:], pr[:], SWAP)
                    nc.vector.tensor_mul(qs_t[:], qs_t[:], tb_t[:, 1, :])
                    nc.vector.tensor_add(dst[:], dst[:], qs_t[:])
                if is_k and norm_d4 is not None:
                    norm_rb[0] = emit_norm_b(norm_d4)
            if norm_d4 is not None:
                emit_norm_c(norm_rb[0])

            # V projection (natural layout [s, f]) into the ones-augmented tiles
            for i in range(4):
                kb = sb * 4 + i
                pr = pp.tile([128, 512], f32, tag="mm")
                for ct in range(8):
                    nc.tensor.matmul(
                        pr[:],
                        xts[ct][:, i * 128 : (i + 1) * 128],
                        wv_t[:, ct, :],
                        start=(ct == 0),
                        stop=(ct == 7),
                    )
                nc.vector.tensor_copy(
                    v_tiles[kb][:, :, 0:64], pr[:].rearrange("p (h d) -> p h d", h=HL)
                )

            if prev_out[0] is not None:
                _emit_out_proj(nc, pp, os_p, wo_t, outp, *prev_out[0])
                prev_out[0] = None

            # attention for this query super-block, score->exp->V software-
            # pipelined. The V accumulation is ROTATED: blocks are summed in
            # order [2, 3, .., n-1, 0, 1], so each head-pair's first V matmul
            # (which must wait for the previous tenant of its PSUM bank to be
            # normalized) fires several score-iterations into the loop --
            # hiding the preceding norm chain's DVE latency from the PE.
            n_kj = 4 * (sb + 1)
            cx_tiles = {}
            norm_st = [None]  # staged mid-attention norm: [d4, rb]
            for hp in range(4):
                cxs = (
                    pc.tile([65, 512], f32, tag="ce", name="ce"),
                    pc.tile([65, 512], f32, tag="co", name="co"),
                )
                exts = {}
                spans = {}

                # rotation requires the start block (kj=2) to cover all 512
                # query columns, which holds only for sb >= 1; sb=0 keeps the
                # natural order (its V matmuls are all tail-emitted anyway)
                rot = n_kj > 4
                start_kj, stop_kj = (2, 1) if rot else (0, n_kj - 1)

                def emit_v(kj):
                    qo, w = spans[kj]
                    for par in (0, 1):
                        nc.tensor.matmul(
                            cxs[par][:, qo : qo + w],
                            v_tiles[kj][:, hp * 2 + par, :],
                            exts[kj][:, par, 0:w],
                            start=(kj == start_kj),
                            stop=(kj == stop_kj),
                        )
                    del exts[kj]

                for kj in range(n_kj):
                    diag = kj >= 4 * sb
                    kjl = kj - 4 * sb
                    w = max(512 - 128 * kjl, 256) if diag else 512
                    qo = 512 - w
                    moff = 384 - (128 * kjl - qo)
                    sbk, col = kj // 4, (kj % 4) * 128
                    spans[kj] = (qo, w)
                    scp = ps.tile([128, 2, 512], f32, tag="sc", name="sc")
                    for par in (0, 1):
                        bp = 64 * par
                        kt_sl = kt_tiles[hp, sbk][bp : bp + 64, col : col + 128]
                        qt_sl = qt_tiles[hp][bp : bp + 64, qo : qo + w]
                        if diag:
                            tw = 128 if kjl < 3 else 256
                            nc.tensor.matmul(
                                scp[:, par, qo : qo + w], kt_sl, qt_sl, start=True, stop=False
                            )
                            nc.tensor.matmul(
                                scp[:, par, qo : qo + tw],
                                ident_t[:],
                                maskT_t[:, moff : moff + tw],
                                start=False,
                                stop=True,
                            )
                        else:
                            nc.tensor.matmul(
                                scp[:, par, :], kt_sl, qt_sl, start=True, stop=True
                            )
                    if rot and kj < 2:
                        # blocks 0/1 are V-accumulated last; their probs
                        # live in dedicated buffers until the tail
                        ext = ex_p.tile(
                            [128, 2, 512], f32r, tag=f"exh{kj}", name="exh", bufs=1
                        )
                    else:
                        ext = ex_p.tile([128, 2, 512], f32r, tag="ex", name="ex")
                    nc.scalar.activation(
                        ext[:, :, 0:w], scp[:, :, qo : qo + w], AF.Exp, scale=0.125
                    )
                    exts[kj] = ext
                    if norm_st[0] is not None:
                        if kj == 1:
                            norm_st[0][1] = emit_norm_b(norm_st[0][0])
                        elif kj == 3:
                            emit_norm_c(norm_st[0][1])
                            norm_st[0] = None
                    if rot:
                        if kj >= 5:
                            emit_v(kj - 3)
                    elif kj >= 2:
                        emit_v(kj - 2)
                if rot:
                    for kj in range(max(2, n_kj - 3), n_kj):
                        emit_v(kj)
                    emit_v(0)
                    emit_v(1)
                else:
                    for kj in range(max(0, n_kj - 2), n_kj):
                        emit_v(kj)
                pending_norm[0] = (hp, cxs, cx_tiles)
                if hp < 3:
                    norm_st[0] = [emit_norm_a(), None]

            # defer: output projection of the PREVIOUS super-block is emitted
            # by the next iteration (after its projections) so its PSUM slots
            # don't gate the next projections; see emit_out_proj below.
            prev_out[0] = (s0, cx_tiles)
        emit_norm()
        if prev_out[0] is not None:
            _emit_out_proj(nc, pp, os_p, wo_t, outp, *prev_out[0])
            prev_out[0] = None


def _emit_out_proj(nc, os_psum, os_p, wo_t, outp, s0, cx_tiles):
    # Wo chunks are the STATIONARY operand ([128,128] -> 107ns LDWEIGHTS,
    # hidden under the 213ns matmul) and ctx the moving one; the output is
    # therefore feature-major [D, S] and the host transposes when gathering.
    from concourse import mybir

    f32 = mybir.dt.float32
    for og in range(2):
        ost = os_p.tile([128, 4, 512], f32, tag="os", name="ost")
        for i in range(4):
            oc = og * 4 + i
            opp = os_psum.tile([128, 512], f32, tag="mm", name="opp")
            for hp in range(4):
                nc.tensor.matmul(
                    opp[:],
                    wo_t[:, hp, oc * 128 : (oc + 1) * 128],
                    cx_tiles[hp][:],
                    start=(hp == 0),
                    stop=(hp == 3),
                )
            nc.vector.tensor_copy(ost[:, i, :], opp[:])
        eng = nc.sync if og == 0 else nc.scalar
        eng.dma_start(
            outp[og * 512 : (og + 1) * 512, s0 : s0 + 512].rearrange(
                "(t p) q -> p t q", p=128
            ),
            ost[:],
        )


def build(S=S_FULL, reps=1, chain=False):
    import concourse.tile as tile
    from concourse import bacc, mybir

    f32 = mybir.dt.float32
    bf16 = mybir.dt.bfloat16
    nc = bacc.Bacc(None, target_bir_lowering=False, debug=False)
    t = {}
    t["xT"] = nc.dram_tensor("xT", [D, S], bf16, kind="ExternalInput")
    t["wqT"] = nc.dram_tensor("wqT", [D, DL], bf16, kind="ExternalInput")
    t["wkT"] = nc.dram_tensor("wkT", [D, DL], bf16, kind="ExternalInput")
    t["wvT"] = nc.dram_tensor("wvT", [D, DL], bf16, kind="ExternalInput")
    t["woT"] = nc.dram_tensor("woT", [DL, D], f32, kind="ExternalInput")
    t["tabs"] = nc.dram_tensor("tabs", [128, 2, S], f32, kind="ExternalInput")
    t["maskT"] = nc.dram_tensor("maskT", [128, 896], mybir.dt.bfloat16, kind="ExternalInput")
    t["ident"] = nc.dram_tensor("ident", [128, 128], mybir.dt.bfloat16, kind="ExternalInput")
    t["outp"] = nc.dram_tensor("outp", [D, S], f32, kind="ExternalOutput")
    if chain:
        t["chain"] = nc.dram_tensor("chain", [128, 128], f32, kind="ExternalInput")
        t["chain_out"] = nc.dram_tensor("chain_out", [128, 128], f32, kind="ExternalOutput")

    with tile.TileContext(nc) as tc:
        _emit(nc, tc, t, S, reps=reps)
        if chain:
            with tc.tile_pool(name="chp", bufs=1) as chp:
                cht = chp.tile([128, 128], mybir.dt.float32, name="cht")
                nc.sync.dma_start(cht[:], t["chain"][:])
                nc.sync.dma_start(t["chain_out"][:], cht[:])
    nc.compile()
    return nc


def prep_inputs(x, Wq, Wk, Wv, Wo, token_positions, S=S_FULL):
    import ml_dtypes

    bf = ml_dtypes.bfloat16
    x = np.asarray(x)
    Wq, Wk, Wv, Wo = (np.asarray(a) for a in (Wq, Wk, Wv, Wo))
    pos = np.asarray(token_positions).astype(np.float64)
    inv = ROPE_THETA ** (-np.arange(0, DK, 2, dtype=np.float64) / DK)  # [32]
    ang = pos[:, None] * inv[None, :]  # [S, 32]
    cos = np.cos(ang).astype(np.float32).T  # [32, S]
    sin = np.sin(ang).astype(np.float32).T
    i_of_p = (np.arange(128) % 64) // 2
    c2 = cos[i_of_p, :]  # [128, S]
    sgn = np.where(np.arange(128) % 2 == 0, -1.0, 1.0).astype(np.float32)
    s2m = sin[i_of_p, :] * sgn[:, None]
    tabs = np.ascontiguousarray(np.stack([c2, s2m], axis=1))  # [128, 2, S]

    maskT = np.where(
        np.arange(896)[None, :] >= np.arange(128)[:, None] + 384, 0.0, NEG
    ).astype(ml_dtypes.bfloat16)
    ident = np.eye(128, dtype=ml_dtypes.bfloat16)

    nb = x.shape[0]
    maps = []
    for c in range(2 * nb):
        b, half = c // 2, c % 2
        rows = slice(half * DL, (half + 1) * DL)
        maps.append(
            {
                "xT": np.ascontiguousarray(x[b].T).astype(bf),
                "wqT": np.ascontiguousarray(Wq[rows].T).astype(bf),
                "wkT": np.ascontiguousarray(Wk[rows].T).astype(bf),
                "wvT": np.ascontiguousarray(Wv[rows].T).astype(bf),
                "woT": np.ascontiguousarray(Wo[:, rows].T),
                "tabs": tabs,
                "maskT": maskT,
                "ident": ident,
            }
        )
    return maps


def kernel(x, Wq, Wk, Wv, Wo, token_positions):
    from concourse.bass_utils import run_bass_kernel_spmd

    if "nc" not in _CACHE:
        _CACHE["nc"] = build()
    maps = prep_inputs(x, Wq, Wk, Wv, Wo, token_positions)
    res = run_bass_kernel_spmd(_CACHE["nc"], maps, list(range(8)))
    out = np.empty((B, S_FULL, D), np.float32)
    for b in range(B):
        out[b] = (res.results[2 * b]["outp"] + res.results[2 * b + 1]["outp"]).T
    return out



# revision 50
# speedup vs baseline: 1.1507x; 1.1507x over previous
"""Causal multi-head self-attention with RoPE on 8 Trainium2 NeuronCores.

Sharding: batch (4) x head-half (2) -> 8 self-contained cores. Each core
computes Q/K/V projections for its 8 heads, RoPE, causal flash-style
attention (scores kept transposed [key, query] so probs feed the V matmul
with no on-device transpose), and a partial output projection over its 512
context features. The two partial outputs per batch are summed on host
(the "all-reduce after output projection" of the tensor-parallel split).

Device layout notes:
- Q/K/V projection matmuls run in bf16 (x and the three weight matrices
  are host-cast): fp32 HIGH mode streams the PE at half rate, bf16 at
  full rate, so the projections take half the PE time. Attention and the
  output projection stay float32r (full fp32 data, fast PE path).
- Softmax: scores are masked additively via a PE-written -1e30 triangle,
  exponentiated without max-subtraction (scores are bounded; verified),
  and the per-query sums come free from a ones-column appended to V.
- RoPE is evaluated as q*cos + swap(q)*(+-sin) where swap is a DVE
  stream_shuffle partition pair-swap and the sign is folded into the sin
  table.
"""

import sys

sys.path.insert(0, "/opt/trn_rl_repo")

import numpy as np

B, S_FULL, D, H = 4, 2048, 1024, 16
DK = 64  # head dim
HL = 8  # heads per core
DL = HL * DK  # 512 local features
ROPE_THETA = 10000.0
NEG = -1.0e30

_CACHE = {}


def _emit(nc, tc, tensors, S, reps=1):
    import concourse.tile as tile  # noqa: F401
    from concourse import mybir
    from contextlib import ExitStack

    f32, f32r = mybir.dt.float32, mybir.dt.float32r
    bf16 = mybir.dt.bfloat16
    AF = mybir.ActivationFunctionType
    SWAP = [i ^ 1 for i in range(32)]
    NSB = S // 512  # query super-blocks
    NKB = S // 128  # key blocks

    xT, wqT, wkT, wvT, woT = (
        tensors["xT"], tensors["wqT"], tensors["wkT"], tensors["wvT"], tensors["woT"],
    )
    tabs, maskT, ident = tensors["tabs"], tensors["maskT"], tensors["ident"]
    outp = tensors["outp"]

    with ExitStack() as ctx:
        const = ctx.enter_context(tc.tile_pool(name="const", bufs=1))
        wres = ctx.enter_context(tc.tile_pool(name="wres", bufs=1))
        xt_p = ctx.enter_context(tc.tile_pool(name="xt", bufs=10))
        tb_p = ctx.enter_context(tc.tile_pool(name="tb", bufs=2))
        kt_p = ctx.enter_context(tc.tile_pool(name="kt", bufs=1))
        vt_p = ctx.enter_context(tc.tile_pool(name="vt", bufs=1))
        qt_p = ctx.enter_context(tc.tile_pool(name="qt", bufs=1))
        qs_p = ctx.enter_context(tc.tile_pool(name="qs", bufs=2))
        ex_p = ctx.enter_context(tc.tile_pool(name="ex", bufs=4))
        cx_p = ctx.enter_context(tc.tile_pool(name="cx", bufs=1))
        rc_p = ctx.enter_context(tc.tile_pool(name="rc", bufs=1))
        rl_p = ctx.enter_context(tc.tile_pool(name="rl", bufs=1))
        os_p = ctx.enter_context(tc.tile_pool(name="os", bufs=2))
        pp = ctx.enter_context(tc.tile_pool(name="pp", bufs=2, space="PSUM"))
        ps = ctx.enter_context(tc.tile_pool(name="ps", bufs=2, space="PSUM"))
        pc = ctx.enter_context(tc.tile_pool(name="pc", bufs=1, space="PSUM"))

        # resident weights (contraction-major), loaded per 128-row chunk (Pool queue)
        woT_r = woT.rearrange("(t p) o -> p t o", p=128)
        # prefetch x tiles and rope tables for the first super-block before
        # the big weight loads so the first projection starts early
        xts0 = []
        for ct in range(8):
            xt_t = xt_p.tile([128, 512], bf16, tag="xt", name="xt0")
            nc.gpsimd.dma_start(xt_t[:], xT[ct * 128 : (ct + 1) * 128, 0:512])
            xts0.append(xt_t)
        tb_t0 = tb_p.tile([128, 2, 512], f32, tag="tb", name="tb0")
        nc.sync.dma_start(tb_t0[:], tabs[:, :, 0:512])
        wq_t = wres.tile([128, 8, DL], bf16, tag="wq")
        wk_t = wres.tile([128, 8, DL], bf16, tag="wk")
        wv_t = wres.tile([128, 8, DL], bf16, tag="wv")
        wo_t = wres.tile([128, 4, D], f32r, tag="wo")
        for ct in range(8):
            nc.sync.dma_start(wk_t[:, ct, :], wkT[ct * 128 : (ct + 1) * 128, :])
        for w_t, wT in ((wq_t, wqT), (wv_t, wvT)):
            for ct in range(8):
                nc.gpsimd.dma_start(w_t[:, ct, :], wT[ct * 128 : (ct + 1) * 128, :])
        # small constants after the weight loads (needed later than W)
        maskT_t = const.tile([128, 896], bf16, tag="maskT")
        nc.sync.dma_start(maskT_t[:], maskT[:])
        ident_t = const.tile([128, 128], bf16, tag="ident")
        nc.sync.dma_start(ident_t[:], ident[:])
        for it in range(4):
            nc.sync.dma_start(
                wo_t[:, it, :], woT_r[:, it, :].bitcast(f32r)
            )

        # persistent K / V buffers
        kt_tiles = {}
        for hp in range(4):
            for sbk in range(NSB):
                kt_tiles[hp, sbk] = kt_p.tile([128, 512], f32r, tag=f"kt{hp}_{sbk}", name=f"kt{hp}_{sbk}")
        v_tiles = {}
        for kb in range(NKB):
            v_tiles[kb] = vt_p.tile([128, HL, 65], f32r, tag=f"v{kb}", name=f"v{kb}")
            nc.vector.memset(v_tiles[kb][:, :, 64:65].bitcast(f32), 1.0)

        prev_out = [None]
        pending_norm = [None]
        next_x = [None]

        def load_x(s0n, name="xt"):
            xts_n = []
            for ct in range(8):
                xt_t = xt_p.tile([128, 512], bf16, tag="xt", name=name)
                eng = nc.gpsimd if ct % 2 == 0 else nc.sync
                eng.dma_start(xt_t[:], xT[ct * 128 : (ct + 1) * 128, s0n : s0n + 512])
                xts_n.append(xt_t)
            tb_n = tb_p.tile([128, 2, 512], f32, tag="tb", name="tb")
            nc.sync.dma_start(tb_n[:], tabs[:, :, s0n : s0n + 512])
            return xts_n, tb_n

        def emit_norm_a():
            # stage A of a head-pair's softmax normalization: evacuate the
            # denominator row to SBUF reshaped [128,4] (16B/partition DMA) so
            # the iterative-divide reciprocal costs 8 lane-elements, then
            # broadcast the reciprocals to 64 partitions on gpsimd. Emits no
            # DVE op that has to WAIT (the recip is stage B) so it never
            # blocks later DVE work queued behind it.
            hp_, cxs_, cxd_ = pending_norm[0]
            den_s = rc_p.tile([1, 2, 512], f32, tag="dens", name="dens")
            d4 = rc_p.tile([128, 2, 4], f32, tag="d4", name="d4")
            for par in (0, 1):
                nc.vector.tensor_copy(den_s[:, par, :], cxs_[par][64:65, :])
                nc.scalar.dma_start(d4[:, par, :], den_s[:, par, :])
            return d4

        def emit_norm_b(d4):
            r4 = rc_p.tile([128, 2, 4], f32, tag="r4", name="r4")
            rc0 = rc_p.tile([1, 512], f32, tag="rc0", name="rc0")
            rc1 = rc_p.tile([1, 512], f32, tag="rc1", name="rc1")
            rb_t = rc_p.tile([64, 2, 512], f32, tag="rb", name="rb")
            nc.vector.reciprocal(r4[:], d4[:])
            for par, rc_t in ((0, rc0), (1, rc1)):
                nc.gpsimd.dma_start(rc_t[:], r4[:, par, :])
                nc.gpsimd.partition_broadcast(rb_t[:, par, :], rc_t[:])
            return rb_t

        def emit_norm_c(rb_t):
            hp_, cxs_, cxd_ = pending_norm[0]
            pending_norm[0] = None
            cxt = cx_p.tile([128, 512], f32r, tag=f"cx{hp_}", name=f"cx{hp_}")
            cxd_[hp_] = cxt
            nc.vector.tensor_mul(cxt[0:64, :], cxs_[0][0:64, :], rb_t[:, 0, :])
            rl_t = rl_p.tile([64, 512], f32r, tag="rl", name="rl")
            nc.vector.tensor_mul(rl_t[:], cxs_[1][0:64, :], rb_t[:, 1, :])
            nc.sync.dma_start(cxt[64:128, :], rl_t[:])

        def emit_norm():
            if pending_norm[0] is None:
                return
            emit_norm_c(emit_norm_b(emit_norm_a()))

        for _rep in range(reps):
         for sb in range(NSB):
            s0 = sb * 512
            if _rep == 0 and sb == 0:
                xts, tb_t = xts0, tb_t0
            else:
                xts, tb_t = load_x(s0)
            # the previous super-block's last norm is flushed in stages woven
            # through this block's projection phase: each stage's ops only
            # execute once their inputs are surely ready, so they never block
            # the DVE/gpsimd queues that feed the rope -> projection pipeline
            norm_d4 = emit_norm_a() if pending_norm[0] is not None else None
            norm_rb = [None]

            # K and Q projections (transposed layout [f, s]) + RoPE
            qt_tiles = {}
            for w_t, is_k in ((wk_t, True), (wq_t, False)):
                for ft in range(4):
                    pr = pp.tile([128, 512], f32, tag="mm")
                    for ct in range(8):
                        nc.tensor.matmul(
                            pr[:],
                            w_t[:, ct, ft * 128 : (ft + 1) * 128],
                            xts[ct][:],
                            start=(ct == 0),
                            stop=(ct == 7),
                        )
                    # rope: dst = pr * cos + pairswap(pr) * (+-sin); both
                    # readers of pr (mul + shuffle) are issued first so the
                    # PSUM slot frees as fast as PE produces the next group
                    if is_k:
                        dst = kt_tiles[ft, sb]
                    else:
                        dst = qt_p.tile([128, 512], f32r, tag=f"qt{ft}", name=f"qt{ft}")
                        qt_tiles[ft] = dst
                    qs_t = qs_p.tile([128, 512], f32, tag="qs")
                    nc.vector.tensor_mul(dst[:], pr[:], tb_t[:, 0, :])
                    nc.vector.stream_shuffle(qs_t[:], pr[:], SWAP)
                    nc.vector.tensor_mul(qs_t[:], qs_t[:], tb_t[:, 1, :])
                    nc.vector.tensor_add(dst[:], dst[:], qs_t[:])
                if is_k and norm_d4 is not None:
                    norm_rb[0] = emit_norm_b(norm_d4)
            if norm_d4 is not None:
                emit_norm_c(norm_rb[0])

            # V projection (natural layout [s, f]) into the ones-augmented tiles
            for i in range(4):
                kb = sb * 4 + i
                pr = pp.tile([128, 512], f32, tag="mm")
                for ct in range(8):
                    nc.tensor.matmul(
                        pr[:],
                        xts[ct][:, i * 128 : (i + 1) * 128],
                        wv_t[:, ct, :],
                        start=(ct == 0),
                        stop=(ct == 7),
                    )
                nc.vector.tensor_copy(
                    v_tiles[kb][:, :, 0:64], pr[:].rearrange("p (h d) -> p h d", h=HL)
                )

            if prev_out[0] is not None:
                _emit_out_proj(nc, pp, os_p, wo_t, outp, *prev_out[0])
                prev_out[0] = None

            # attention for this query super-block, score->exp->V software-
            # pipelined. The V accumulation is ROTATED: blocks are summed in
            # order [2, 3, .., n-1, 0, 1], so each head-pair's first V matmul
            # (which must wait for the previous tenant of its PSUM bank to be
            # normalized) fires several score-iterations into the loop --
            # hiding the preceding norm chain's DVE latency from the PE.
            n_kj = 4 * (sb + 1)
            cx_tiles = {}
            norm_st = [None]  # staged mid-attention norm: [d4, rb]
            for hp in range(4):
                cxs = (
                    pc.tile([65, 512], f32, tag="ce", name="ce"),
                    pc.tile([65, 512], f32, tag="co", name="co"),
                )
                exts = {}
                spans = {}

                # rotation requires the start block (kj=2) to cover all 512
                # query columns, which holds only for sb >= 1; sb=0 keeps the
                # natural order (its V matmuls are all tail-emitted anyway)
                rot = n_kj > 4
                start_kj, stop_kj = (2, 1) if rot else (0, n_kj - 1)

                def emit_v(kj):
                    qo, w = spans[kj]
                    for par in (0, 1):
                        nc.tensor.matmul(
                            cxs[par][:, qo : qo + w],
                            v_tiles[kj][:, hp * 2 + par, :],
                            exts[kj][:, par, 0:w],
                            start=(kj == start_kj),
                            stop=(kj == stop_kj),
                        )
                    del exts[kj]

                for kj in range(n_kj):
                    diag = kj >= 4 * sb
                    kjl = kj - 4 * sb
                    w = max(512 - 128 * kjl, 256) if diag else 512
                    qo = 512 - w
                    moff = 384 - (128 * kjl - qo)
                    sbk, col = kj // 4, (kj % 4) * 128
                    spans[kj] = (qo, w)
                    scp = ps.tile([128, 2, 512], f32, tag="sc", name="sc")
                    for par in (0, 1):
                        bp = 64 * par
                        kt_sl = kt_tiles[hp, sbk][bp : bp + 64, col : col + 128]
                        qt_sl = qt_tiles[hp][bp : bp + 64, qo : qo + w]
                        if diag:
                            tw = 128 if kjl < 3 else 256
                            nc.tensor.matmul(
                                scp[:, par, qo : qo + w], kt_sl, qt_sl, start=True, stop=False
                            )
                            nc.tensor.matmul(
                                scp[:, par, qo : qo + tw],
                                ident_t[:],
                                maskT_t[:, moff : moff + tw],
                                start=False,
                                stop=True,
                            )
                        else:
                            nc.tensor.matmul(
                                scp[:, par, :], kt_sl, qt_sl, start=True, stop=True
                            )
                    if rot and kj < 2:
                        # blocks 0/1 are V-accumulated last; their probs
                        # live in dedicated buffers until the tail
                        ext = ex_p.tile(
                            [128, 2, 512], f32r, tag=f"exh{kj}", name="exh", bufs=1
                        )
                    else:
                        ext = ex_p.tile([128, 2, 512], f32r, tag="ex", name="ex")
                    nc.scalar.activation(
                        ext[:, :, 0:w], scp[:, :, qo : qo + w], AF.Exp, scale=0.125
                    )
                    exts[kj] = ext
                    if norm_st[0] is not None:
                        if kj == 1:
                            norm_st[0][1] = emit_norm_b(norm_st[0][0])
                        elif kj == 3:
                            emit_norm_c(norm_st[0][1])
                            norm_st[0] = None
                    if rot:
                        if kj >= 5:
                            emit_v(kj - 3)
                    elif kj >= 2:
                        emit_v(kj - 2)
                if rot:
                    for kj in range(max(2, n_kj - 3), n_kj):
                        emit_v(kj)
                    emit_v(0)
                    emit_v(1)
                else:
                    for kj in range(max(0, n_kj - 2), n_kj):
                        emit_v(kj)
                pending_norm[0] = (hp, cxs, cx_tiles)
                if hp < 3:
                    norm_st[0] = [emit_norm_a(), None]

            # defer: output projection of the PREVIOUS super-block is emitted
            # by the next iteration (after its projections) so its PSUM slots
            # don't gate the next projections; see emit_out_proj below.
            prev_out[0] = (s0, cx_tiles)
        emit_norm()
        if prev_out[0] is not None:
            _emit_out_proj(nc, pp, os_p, wo_t, outp, *prev_out[0])
            prev_out[0] = None


def _emit_out_proj(nc, os_psum, os_p, wo_t, outp, s0, cx_tiles):
    # Wo chunks are the STATIONARY operand ([128,128] -> 107ns LDWEIGHTS,
    # hidden under the 213ns matmul) and ctx the moving one; the output is
    # therefore feature-major [D, S] and the host transposes when gathering.
    from concourse import mybir

    f32 = mybir.dt.float32
    for og in range(2):
        ost = os_p.tile([128, 4, 512], f32, tag="os", name="ost")
        for i in range(4):
            oc = og * 4 + i
            opp = os_psum.tile([128, 512], f32, tag="mm", name="opp")
            for hp in range(4):
                nc.tensor.matmul(
                    opp[:],
                    wo_t[:, hp, oc * 128 : (oc + 1) * 128],
                    cx_tiles[hp][:],
                    start=(hp == 0),
                    stop=(hp == 3),
                )
            nc.vector.tensor_copy(ost[:, i, :], opp[:])
        eng = nc.sync if og == 0 else nc.scalar
        eng.dma_start(
            outp[og * 512 : (og + 1) * 512, s0 : s0 + 512].rearrange(
                "(t p) q -> p t q", p=128
            ),
            ost[:],
        )


def build(S=S_FULL, reps=1, chain=False):
    import concourse.tile as tile
    from concourse import bacc, mybir

    f32 = mybir.dt.float32
    bf16 = mybir.dt.bfloat16
    nc = bacc.Bacc(None, target_bir_lowering=False, debug=False)
    t = {}
    t["xT"] = nc.dram_tensor("xT", [D, S], bf16, kind="ExternalInput")
    t["wqT"] = nc.dram_tensor("wqT", [D, DL], bf16, kind="ExternalInput")
    t["wkT"] = nc.dram_tensor("wkT", [D, DL], bf16, kind="ExternalInput")
    t["wvT"] = nc.dram_tensor("wvT", [D, DL], bf16, kind="ExternalInput")
    t["woT"] = nc.dram_tensor("woT", [DL, D], f32, kind="ExternalInput")
    t["tabs"] = nc.dram_tensor("tabs", [128, 2, S], f32, kind="ExternalInput")
    t["maskT"] = nc.dram_tensor("maskT", [128, 896], mybir.dt.bfloat16, kind="ExternalInput")
    t["ident"] = nc.dram_tensor("ident", [128, 128], mybir.dt.bfloat16, kind="ExternalInput")
    t["outp"] = nc.dram_tensor("outp", [D, S], f32, kind="ExternalOutput")
    if chain:
        t["chain"] = nc.dram_tensor("chain", [128, 128], f32, kind="ExternalInput")
        t["chain_out"] = nc.dram_tensor("chain_out", [128, 128], f32, kind="ExternalOutput")

    with tile.TileContext(nc) as tc:
        _emit(nc, tc, t, S, reps=reps)
        if chain:
            with tc.tile_pool(name="chp", bufs=1) as chp:
                cht = chp.tile([128, 128], mybir.dt.float32, name="cht")
                nc.sync.dma_start(cht[:], t["chain"][:])
                nc.sync.dma_start(t["chain_out"][:], cht[:])
    nc.compile()
    return nc


def prep_inputs(x, Wq, Wk, Wv, Wo, token_positions, S=S_FULL):
    import ml_dtypes

    bf = ml_dtypes.bfloat16
    x = np.asarray(x)
    Wq, Wk, Wv, Wo = (np.asarray(a) for a in (Wq, Wk, Wv, Wo))
    pos = np.asarray(token_positions).astype(np.float64)
    inv = ROPE_THETA ** (-np.arange(0, DK, 2, dtype=np.float64) / DK)  # [32]
    ang = pos[:, None] * inv[None, :]  # [S, 32]
    cos = np.cos(ang).astype(np.float32).T  # [32, S]
    sin = np.sin(ang).astype(np.float32).T
    i_of_p = (np.arange(128) % 64) // 2
    c2 = cos[i_of_p, :]  # [128, S]
    sgn = np.where(np.arange(128) % 2 == 0, -1.0, 1.0).astype(np.float32)
    s2m = sin[i_of_p, :] * sgn[:, None]
    tabs = np.ascontiguousarray(np.stack([c2, s2m], axis=1))  # [128, 2, S]

    maskT = np.where(
        np.arange(896)[None, :] >= np.arange(128)[:, None] + 384, 0.0, NEG
    ).astype(ml_dtypes.bfloat16)
    ident = np.eye(128, dtype=ml_dtypes.bfloat16)

    nb = x.shape[0]
    maps = []
    for c in range(2 * nb):
        b, half = c // 2, c % 2
        rows = slice(half * DL, (half + 1) * DL)
        maps.append(
            {
                "xT": np.ascontiguousarray(x[b].T).astype(bf),
                "wqT": np.ascontiguousarray(Wq[rows].T).astype(bf),
                "wkT": np.ascontiguousarray(Wk[rows].T).astype(bf),
                "wvT": np.ascontiguousarray(Wv[rows].T).astype(bf),
                "woT": np.ascontiguousarray(Wo[:, rows].T),
                "tabs": tabs,
                "maskT": maskT,
                "ident": ident,
            }
        )
    return maps


def kernel(x, Wq, Wk, Wv, Wo, token_positions):
    from concourse.bass_utils import run_bass_kernel_spmd

    if "nc" not in _CACHE:
        _CACHE["nc"] = build()
    maps = prep_inputs(x, Wq, Wk, Wv, Wo, token_positions)
    res = run_bass_kernel_spmd(_CACHE["nc"], maps, list(range(8)))
    out = np.empty((B, S_FULL, D), np.float32)
    for b in range(B):
        out[b] = (res.results[2 * b]["outp"] + res.results[2 * b + 1]["outp"]).T
    return out



# revision 53
# speedup vs baseline: 1.2694x; 1.1031x over previous
"""Causal multi-head self-attention with RoPE on 8 Trainium2 NeuronCores.

Sharding: batch (4) x head-half (2) -> 8 self-contained cores. Each core
computes Q/K/V projections for its 8 heads, RoPE, causal flash-style
attention (scores kept transposed [key, query] so probs feed the V matmul
with no on-device transpose), and a partial output projection over its 512
context features. The two partial outputs per batch are summed on host
(the "all-reduce after output projection" of the tensor-parallel split).

Device layout notes:
- Q/K/V projection matmuls run in bf16 (x and the three weight matrices
  are host-cast): fp32 HIGH mode streams the PE at half rate, bf16 at
  full rate, so the projections take half the PE time. Attention and the
  output projection stay float32r (full fp32 data, fast PE path).
- Softmax: scores are masked additively via a PE-written -1e30 triangle,
  exponentiated without max-subtraction (scores are bounded; verified),
  and the per-query sums come free from a ones-column appended to V.
- RoPE is evaluated as q*cos + swap(q)*(+-sin) where swap is a DVE
  stream_shuffle partition pair-swap and the sign is folded into the sin
  table.
"""

import sys

sys.path.insert(0, "/opt/trn_rl_repo")

import numpy as np

B, S_FULL, D, H = 4, 2048, 1024, 16
DK = 64  # head dim
HL = 8  # heads per core
DL = HL * DK  # 512 local features
ROPE_THETA = 10000.0
NEG = -1.0e30

_CACHE = {}


def _emit(nc, tc, tensors, S, reps=1):
    import concourse.tile as tile  # noqa: F401
    from concourse import mybir
    from contextlib import ExitStack

    f32, f32r = mybir.dt.float32, mybir.dt.float32r
    bf16 = mybir.dt.bfloat16
    AF = mybir.ActivationFunctionType
    SWAP = [i ^ 1 for i in range(32)]
    NSB = S // 512  # query super-blocks
    NKB = S // 128  # key blocks

    xT, wqT, wkT, wvT, woT = (
        tensors["xT"], tensors["wqT"], tensors["wkT"], tensors["wvT"], tensors["woT"],
    )
    tabs, maskT, ident = tensors["tabs"], tensors["maskT"], tensors["ident"]
    outp = tensors["outp"]

    with ExitStack() as ctx:
        const = ctx.enter_context(tc.tile_pool(name="const", bufs=1))
        wres = ctx.enter_context(tc.tile_pool(name="wres", bufs=1))
        xt_p = ctx.enter_context(tc.tile_pool(name="xt", bufs=10))
        tb_p = ctx.enter_context(tc.tile_pool(name="tb", bufs=2))
        kt_p = ctx.enter_context(tc.tile_pool(name="kt", bufs=1))
        vt_p = ctx.enter_context(tc.tile_pool(name="vt", bufs=1))
        qt_p = ctx.enter_context(tc.tile_pool(name="qt", bufs=1))
        qs_p = ctx.enter_context(tc.tile_pool(name="qs", bufs=2))
        ex_p = ctx.enter_context(tc.tile_pool(name="ex", bufs=5))
        cx_p = ctx.enter_context(tc.tile_pool(name="cx", bufs=1))
        rc_p = ctx.enter_context(tc.tile_pool(name="rc", bufs=1))
        rl_p = ctx.enter_context(tc.tile_pool(name="rl", bufs=1))
        os_p = ctx.enter_context(tc.tile_pool(name="os", bufs=2))
        pp = ctx.enter_context(tc.tile_pool(name="pp", bufs=2, space="PSUM"))
        ps = ctx.enter_context(tc.tile_pool(name="ps", bufs=2, space="PSUM"))
        pc = ctx.enter_context(tc.tile_pool(name="pc", bufs=1, space="PSUM"))

        # resident weights (contraction-major), loaded per 128-row chunk (Pool queue)
        woT_r = woT.rearrange("(t p) o -> p t o", p=128)
        # prefetch x tiles and rope tables for the first super-block before
        # the big weight loads so the first projection starts early
        xts0 = []
        for ct in range(8):
            xt_t = xt_p.tile([128, 512], bf16, tag="xt", name="xt0")
            nc.gpsimd.dma_start(xt_t[:], xT[ct * 128 : (ct + 1) * 128, 0:512])
            xts0.append(xt_t)
        tb_t0 = tb_p.tile([128, 2, 512], f32, tag="tb", name="tb0")
        nc.sync.dma_start(tb_t0[:], tabs[:, :, 0:512])
        wq_t = wres.tile([128, 8, DL], bf16, tag="wq")
        wk_t = wres.tile([128, 8, DL], bf16, tag="wk")
        wv_t = wres.tile([128, 8, DL], bf16, tag="wv")
        wo_t = wres.tile([128, 4, D], f32r, tag="wo")
        for ct in range(8):
            nc.sync.dma_start(wk_t[:, ct, :], wkT[ct * 128 : (ct + 1) * 128, :])
        for w_t, wT in ((wq_t, wqT), (wv_t, wvT)):
            for ct in range(8):
                nc.gpsimd.dma_start(w_t[:, ct, :], wT[ct * 128 : (ct + 1) * 128, :])
        # small constants after the weight loads (needed later than W)
        maskT_t = const.tile([128, 896], bf16, tag="maskT")
        nc.sync.dma_start(maskT_t[:], maskT[:])
        ident_t = const.tile([128, 128], bf16, tag="ident")
        nc.sync.dma_start(ident_t[:], ident[:])
        for it in range(4):
            nc.sync.dma_start(
                wo_t[:, it, :], woT_r[:, it, :].bitcast(f32r)
            )

        # persistent K / V buffers
        kt_tiles = {}
        for hp in range(4):
            for sbk in range(NSB):
                kt_tiles[hp, sbk] = kt_p.tile([128, 512], f32r, tag=f"kt{hp}_{sbk}", name=f"kt{hp}_{sbk}")
        v_tiles = {}
        for kb in range(NKB):
            v_tiles[kb] = vt_p.tile([128, HL, 65], f32r, tag=f"v{kb}", name=f"v{kb}")
            nc.vector.memset(v_tiles[kb][:, :, 64:65].bitcast(f32), 1.0)

        prev_out = [None]
        pending_norm = [None]
        next_x = [None]

        def load_x(s0n, name="xt"):
            xts_n = []
            for ct in range(8):
                xt_t = xt_p.tile([128, 512], bf16, tag="xt", name=name)
                eng = nc.gpsimd if ct % 2 == 0 else nc.sync
                eng.dma_start(xt_t[:], xT[ct * 128 : (ct + 1) * 128, s0n : s0n + 512])
                xts_n.append(xt_t)
            tb_n = tb_p.tile([128, 2, 512], f32, tag="tb", name="tb")
            nc.sync.dma_start(tb_n[:], tabs[:, :, s0n : s0n + 512])
            return xts_n, tb_n

        def emit_norm_a():
            # stage A of a head-pair's softmax normalization: evacuate the
            # denominator row to SBUF reshaped [128,4] (16B/partition DMA) so
            # the iterative-divide reciprocal costs 8 lane-elements, then
            # broadcast the reciprocals to 64 partitions on gpsimd. Emits no
            # DVE op that has to WAIT (the recip is stage B) so it never
            # blocks later DVE work queued behind it.
            hp_, cxs_, cxd_ = pending_norm[0]
            den_s = rc_p.tile([1, 2, 512], f32, tag="dens", name="dens")
            d4 = rc_p.tile([128, 2, 4], f32, tag="d4", name="d4")
            for par in (0, 1):
                nc.vector.tensor_copy(den_s[:, par, :], cxs_[par][64:65, :])
                nc.scalar.dma_start(d4[:, par, :], den_s[:, par, :])
            return d4

        def emit_norm_b(d4):
            r4 = rc_p.tile([128, 2, 4], f32, tag="r4", name="r4")
            rc0 = rc_p.tile([1, 512], f32, tag="rc0", name="rc0")
            rc1 = rc_p.tile([1, 512], f32, tag="rc1", name="rc1")
            rb_t = rc_p.tile([64, 2, 512], f32, tag="rb", name="rb")
            nc.vector.reciprocal(r4[:], d4[:])
            for par, rc_t in ((0, rc0), (1, rc1)):
                nc.gpsimd.dma_start(rc_t[:], r4[:, par, :])
                nc.gpsimd.partition_broadcast(rb_t[:, par, :], rc_t[:])
            return rb_t

        def emit_norm_c(rb_t):
            hp_, cxs_, cxd_ = pending_norm[0]
            pending_norm[0] = None
            cxt = cx_p.tile([128, 512], f32r, tag=f"cx{hp_}", name=f"cx{hp_}")
            cxd_[hp_] = cxt
            nc.vector.tensor_mul(cxt[0:64, :], cxs_[0][0:64, :], rb_t[:, 0, :])
            rl_t = rl_p.tile([64, 512], f32r, tag="rl", name="rl")
            nc.vector.tensor_mul(rl_t[:], cxs_[1][0:64, :], rb_t[:, 1, :])
            nc.sync.dma_start(cxt[64:128, :], rl_t[:])

        def emit_norm():
            if pending_norm[0] is None:
                return
            emit_norm_c(emit_norm_b(emit_norm_a()))

        for _rep in range(reps):
         for sb in range(NSB):
            s0 = sb * 512
            if _rep == 0 and sb == 0:
                xts, tb_t = xts0, tb_t0
            else:
                xts, tb_t = load_x(s0)
            # the previous super-block's last norm is flushed in stages woven
            # through this block's projection phase: each stage's ops only
            # execute once their inputs are surely ready, so they never block
            # the DVE/gpsimd queues that feed the rope -> projection pipeline
            norm_d4 = emit_norm_a() if pending_norm[0] is not None else None
            norm_rb = [None]

            # K and Q projections (transposed layout [f, s]) + RoPE
            qt_tiles = {}
            for w_t, is_k in ((wk_t, True), (wq_t, False)):
                for ft in range(4):
                    pr = pp.tile([128, 512], f32, tag="mm")
                    for ct in range(8):
                        nc.tensor.matmul(
                            pr[:],
                            w_t[:, ct, ft * 128 : (ft + 1) * 128],
                            xts[ct][:],
                            start=(ct == 0),
                            stop=(ct == 7),
                        )
                    # rope: dst = pr * cos + pairswap(pr) * (+-sin); both
                    # readers of pr (mul + shuffle) are issued first so the
                    # PSUM slot frees as fast as PE produces the next group
                    if is_k:
                        dst = kt_tiles[ft, sb]
                    else:
                        dst = qt_p.tile([128, 512], f32r, tag=f"qt{ft}", name=f"qt{ft}")
                        qt_tiles[ft] = dst
                    qs_t = qs_p.tile([128, 512], f32, tag="qs")
                    nc.vector.tensor_mul(dst[:], pr[:], tb_t[:, 0, :])
                    nc.vector.stream_shuffle(qs_t[:], pr[:], SWAP)
                    nc.vector.tensor_mul(qs_t[:], qs_t[:], tb_t[:, 1, :])
                    nc.vector.tensor_add(dst[:], dst[:], qs_t[:])
                if is_k and norm_d4 is not None:
                    norm_rb[0] = emit_norm_b(norm_d4)
            if norm_d4 is not None:
                emit_norm_c(norm_rb[0])

            # V projection (natural layout [s, f]) into the ones-augmented tiles
            for i in range(4):
                kb = sb * 4 + i
                pr = pp.tile([128, 512], f32, tag="mm")
                for ct in range(8):
                    nc.tensor.matmul(
                        pr[:],
                        xts[ct][:, i * 128 : (i + 1) * 128],
                        wv_t[:, ct, :],
                        start=(ct == 0),
                        stop=(ct == 7),
                    )
                nc.vector.tensor_copy(
                    v_tiles[kb][:, :, 0:64], pr[:].rearrange("p (h d) -> p h d", h=HL)
                )

            if prev_out[0] is not None:
                _emit_out_proj(nc, pp, os_p, wo_t, outp, *prev_out[0])
                prev_out[0] = None

            # attention for this query super-block, score->exp->V software-
            # pipelined. The V accumulation is ROTATED: blocks are summed in
            # order [2, 3, .., n-1, 0, 1], so each head-pair's first V matmul
            # (which must wait for the previous tenant of its PSUM bank to be
            # normalized) fires several score-iterations into the loop --
            # hiding the preceding norm chain's DVE latency from the PE.
            n_kj = 4 * (sb + 1)
            cx_tiles = {}
            norm_st = [None]  # staged mid-attention norm: [d4, rb]
            for hp in range(4):
                cxs = (
                    pc.tile([65, 512], f32, tag="ce", name="ce"),
                    pc.tile([65, 512], f32, tag="co", name="co"),
                )
                exts = {}
                spans = {}

                # rotation requires the start block (kj=2) to cover all 512
                # query columns, which holds only for sb >= 1; sb=0 keeps the
                # natural order (its V matmuls are all tail-emitted anyway)
                rot = n_kj > 4
                start_kj, stop_kj = (2, 1) if rot else (0, n_kj - 1)

                def emit_v(kj):
                    qo, w = spans[kj]
                    for par in (0, 1):
                        nc.tensor.matmul(
                            cxs[par][:, qo : qo + w],
                            v_tiles[kj][:, hp * 2 + par, :],
                            exts[kj][:, par, 0:w],
                            start=(kj == start_kj),
                            stop=(kj == stop_kj),
                        )
                    del exts[kj]

                for kj in range(n_kj):
                    diag = kj >= 4 * sb
                    kjl = kj - 4 * sb
                    w = max(512 - 128 * kjl, 256) if diag else 512
                    qo = 512 - w
                    moff = 384 - (128 * kjl - qo)
                    sbk, col = kj // 4, (kj % 4) * 128
                    spans[kj] = (qo, w)
                    scp = ps.tile([128, 2, 512], f32, tag="sc", name="sc")
                    # both 64-contraction score matmuls first, then (diag
                    # only) both 128-contraction mask matmuls: groups the PE
                    # row-tiling modes to halve array-drain mode switches
                    for par in (0, 1):
                        bp = 64 * par
                        kt_sl = kt_tiles[hp, sbk][bp : bp + 64, col : col + 128]
                        qt_sl = qt_tiles[hp][bp : bp + 64, qo : qo + w]
                        nc.tensor.matmul(
                            scp[:, par, qo : qo + w],
                            kt_sl,
                            qt_sl,
                            start=True,
                            stop=not diag,
                        )
                    if diag:
                        tw = 128 if kjl < 3 else 256
                        for par in (0, 1):
                            nc.tensor.matmul(
                                scp[:, par, qo : qo + tw],
                                ident_t[:],
                                maskT_t[:, moff : moff + tw],
                                start=False,
                                stop=True,
                            )
                    if rot and kj < 2:
                        # blocks 0/1 are V-accumulated last; their probs
                        # live in dedicated buffers until the tail
                        ext = ex_p.tile(
                            [128, 2, 512], f32r, tag=f"exh{kj}", name="exh", bufs=1
                        )
                    else:
                        ext = ex_p.tile([128, 2, 512], f32r, tag="ex", name="ex")
                    nc.scalar.activation(
                        ext[:, :, 0:w], scp[:, :, qo : qo + w], AF.Exp, scale=0.125
                    )
                    exts[kj] = ext
                    if norm_st[0] is not None:
                        if kj == 1:
                            norm_st[0][1] = emit_norm_b(norm_st[0][0])
                        elif kj == 3:
                            emit_norm_c(norm_st[0][1])
                            norm_st[0] = None
                    if rot:
                        # V blocks are emitted in PAIRS at even kj so the
                        # four 128-contraction matmuls run back-to-back
                        # (one row-tiling mode switch instead of two)
                        if kj >= 6 and kj % 2 == 0:
                            emit_v(kj - 4)
                            emit_v(kj - 3)
                    elif kj >= 2:
                        emit_v(kj - 2)
                if rot:
                    for kj in range(max(2, n_kj - 4), n_kj):
                        emit_v(kj)
                    emit_v(0)
                    emit_v(1)
                else:
                    for kj in range(max(0, n_kj - 2), n_kj):
                        emit_v(kj)
                pending_norm[0] = (hp, cxs, cx_tiles)
                if hp < 3:
                    norm_st[0] = [emit_norm_a(), None]

            # defer: output projection of the PREVIOUS super-block is emitted
            # by the next iteration (after its projections) so its PSUM slots
            # don't gate the next projections; see emit_out_proj below.
            prev_out[0] = (s0, cx_tiles)
        emit_norm()
        if prev_out[0] is not None:
            _emit_out_proj(nc, pp, os_p, wo_t, outp, *prev_out[0])
            prev_out[0] = None


def _emit_out_proj(nc, os_psum, os_p, wo_t, outp, s0, cx_tiles):
    # Wo chunks are the STATIONARY operand ([128,128] -> 107ns LDWEIGHTS,
    # hidden under the 213ns matmul) and ctx the moving one; the output is
    # therefore feature-major [D, S] and the host transposes when gathering.
    from concourse import mybir

    f32 = mybir.dt.float32
    for og in range(2):
        ost = os_p.tile([128, 4, 512], f32, tag="os", name="ost")
        for i in range(4):
            oc = og * 4 + i
            opp = os_psum.tile([128, 512], f32, tag="mm", name="opp")
            for hp in range(4):
                nc.tensor.matmul(
                    opp[:],
                    wo_t[:, hp, oc * 128 : (oc + 1) * 128],
                    cx_tiles[hp][:],
                    start=(hp == 0),
                    stop=(hp == 3),
                )
            nc.vector.tensor_copy(ost[:, i, :], opp[:])
        eng = nc.sync if og == 0 else nc.scalar
        eng.dma_start(
            outp[og * 512 : (og + 1) * 512, s0 : s0 + 512].rearrange(
                "(t p) q -> p t q", p=128
            ),
            ost[:],
        )


def build(S=S_FULL, reps=1, chain=False):
    import concourse.tile as tile
    from concourse import bacc, mybir

    f32 = mybir.dt.float32
    bf16 = mybir.dt.bfloat16
    nc = bacc.Bacc(None, target_bir_lowering=False, debug=False)
    t = {}
    t["xT"] = nc.dram_tensor("xT", [D, S], bf16, kind="ExternalInput")
    t["wqT"] = nc.dram_tensor("wqT", [D, DL], bf16, kind="ExternalInput")
    t["wkT"] = nc.dram_tensor("wkT", [D, DL], bf16, kind="ExternalInput")
    t["wvT"] = nc.dram_tensor("wvT", [D, DL], bf16, kind="ExternalInput")
    t["woT"] = nc.dram_tensor("woT", [DL, D], f32, kind="ExternalInput")
    t["tabs"] = nc.dram_tensor("tabs", [128, 2, S], f32, kind="ExternalInput")
    t["maskT"] = nc.dram_tensor("maskT", [128, 896], mybir.dt.bfloat16, kind="ExternalInput")
    t["ident"] = nc.dram_tensor("ident", [128, 128], mybir.dt.bfloat16, kind="ExternalInput")
    t["outp"] = nc.dram_tensor("outp", [D, S], f32, kind="ExternalOutput")
    if chain:
        t["chain"] = nc.dram_tensor("chain", [128, 128], f32, kind="ExternalInput")
        t["chain_out"] = nc.dram_tensor("chain_out", [128, 128], f32, kind="ExternalOutput")

    with tile.TileContext(nc) as tc:
        _emit(nc, tc, t, S, reps=reps)
        if chain:
            with tc.tile_pool(name="chp", bufs=1) as chp:
                cht = chp.tile([128, 128], mybir.dt.float32, name="cht")
                nc.sync.dma_start(cht[:], t["chain"][:])
                nc.sync.dma_start(t["chain_out"][:], cht[:])
    nc.compile()
    return nc


def prep_inputs(x, Wq, Wk, Wv, Wo, token_positions, S=S_FULL):
    import ml_dtypes

    bf = ml_dtypes.bfloat16
    x = np.asarray(x)
    Wq, Wk, Wv, Wo = (np.asarray(a) for a in (Wq, Wk, Wv, Wo))
    pos = np.asarray(token_positions).astype(np.float64)
    inv = ROPE_THETA ** (-np.arange(0, DK, 2, dtype=np.float64) / DK)  # [32]
    ang = pos[:, None] * inv[None, :]  # [S, 32]
    cos = np.cos(ang).astype(np.float32).T  # [32, S]
    sin = np.sin(ang).astype(np.float32).T
    i_of_p = (np.arange(128) % 64) // 2
    c2 = cos[i_of_p, :]  # [128, S]
    sgn = np.where(np.arange(128) % 2 == 0, -1.0, 1.0).astype(np.float32)
    s2m = sin[i_of_p, :] * sgn[:, None]
    tabs = np.ascontiguousarray(np.stack([c2, s2m], axis=1))  # [128, 2, S]

    maskT = np.where(
        np.arange(896)[None, :] >= np.arange(128)[:, None] + 384, 0.0, NEG
    ).astype(ml_dtypes.bfloat16)
    ident = np.eye(128, dtype=ml_dtypes.bfloat16)

    nb = x.shape[0]
    maps = []
    for c in range(2 * nb):
        b, half = c // 2, c % 2
        rows = slice(half * DL, (half + 1) * DL)
        maps.append(
            {
                "xT": np.ascontiguousarray(x[b].T).astype(bf),
                "wqT": np.ascontiguousarray(Wq[rows].T).astype(bf),
                "wkT": np.ascontiguousarray(Wk[rows].T).astype(bf),
                "wvT": np.ascontiguousarray(Wv[rows].T).astype(bf),
                "woT": np.ascontiguousarray(Wo[:, rows].T),
                "tabs": tabs,
                "maskT": maskT,
                "ident": ident,
            }
        )
    return maps


def kernel(x, Wq, Wk, Wv, Wo, token_positions):
    from concourse.bass_utils import run_bass_kernel_spmd

    if "nc" not in _CACHE:
        _CACHE["nc"] = build()
    maps = prep_inputs(x, Wq, Wk, Wv, Wo, token_positions)
    res = run_bass_kernel_spmd(_CACHE["nc"], maps, list(range(8)))
    out = np.empty((B, S_FULL, D), np.float32)
    for b in range(B):
        out[b] = (res.results[2 * b]["outp"] + res.results[2 * b + 1]["outp"]).T
    return out



# revision 58
# speedup vs baseline: 1.2841x; 1.0116x over previous
"""Causal multi-head self-attention with RoPE on 8 Trainium2 NeuronCores.

Sharding: batch (4) x head-half (2) -> 8 self-contained cores. Each core
computes Q/K/V projections for its 8 heads, RoPE, causal flash-style
attention (scores kept transposed [key, query] so probs feed the V matmul
with no on-device transpose), and a partial output projection over its 512
context features. The two partial outputs per batch are summed on host
(the "all-reduce after output projection" of the tensor-parallel split).

Device layout notes:
- Q/K/V projection matmuls run in bf16 (x and the three weight matrices
  are host-cast): fp32 HIGH mode streams the PE at half rate, bf16 at
  full rate, so the projections take half the PE time. Attention and the
  output projection stay float32r (full fp32 data, fast PE path).
- Softmax: scores are masked additively via a PE-written -1e30 triangle,
  exponentiated without max-subtraction (scores are bounded; verified),
  and the per-query sums come free from a ones-column appended to V.
- RoPE is evaluated as q*cos + swap(q)*(+-sin) where swap is a DVE
  stream_shuffle partition pair-swap and the sign is folded into the sin
  table.
"""

import sys

sys.path.insert(0, "/opt/trn_rl_repo")

import numpy as np

B, S_FULL, D, H = 4, 2048, 1024, 16
DK = 64  # head dim
HL = 8  # heads per core
DL = HL * DK  # 512 local features
ROPE_THETA = 10000.0
NEG = -1.0e30

_CACHE = {}


def _emit(nc, tc, tensors, S, reps=1):
    import concourse.tile as tile  # noqa: F401
    from concourse import mybir
    from contextlib import ExitStack

    f32, f32r = mybir.dt.float32, mybir.dt.float32r
    bf16 = mybir.dt.bfloat16
    AF = mybir.ActivationFunctionType
    SWAP = [i ^ 1 for i in range(32)]
    NSB = S // 512  # query super-blocks
    NKB = S // 128  # key blocks

    xT, wqT, wkT, wvT, woT = (
        tensors["xT"], tensors["wqT"], tensors["wkT"], tensors["wvT"], tensors["woT"],
    )
    tabs, maskT, ident = tensors["tabs"], tensors["maskT"], tensors["ident"]
    outp = tensors["outp"]

    with ExitStack() as ctx:
        const = ctx.enter_context(tc.tile_pool(name="const", bufs=1))
        wres = ctx.enter_context(tc.tile_pool(name="wres", bufs=1))
        xt_p = ctx.enter_context(tc.tile_pool(name="xt", bufs=10))
        tb_p = ctx.enter_context(tc.tile_pool(name="tb", bufs=2))
        kt_p = ctx.enter_context(tc.tile_pool(name="kt", bufs=1))
        vt_p = ctx.enter_context(tc.tile_pool(name="vt", bufs=1))
        qt_p = ctx.enter_context(tc.tile_pool(name="qt", bufs=1))
        qs_p = ctx.enter_context(tc.tile_pool(name="qs", bufs=2))
        ex_p = ctx.enter_context(tc.tile_pool(name="ex", bufs=6))
        cx_p = ctx.enter_context(tc.tile_pool(name="cx", bufs=1))
        rc_p = ctx.enter_context(tc.tile_pool(name="rc", bufs=1))
        rl_p = ctx.enter_context(tc.tile_pool(name="rl", bufs=1))
        os_p = ctx.enter_context(tc.tile_pool(name="os", bufs=2))
        pp = ctx.enter_context(tc.tile_pool(name="pp", bufs=2, space="PSUM"))
        ps = ctx.enter_context(tc.tile_pool(name="ps", bufs=2, space="PSUM"))
        pc = ctx.enter_context(tc.tile_pool(name="pc", bufs=1, space="PSUM"))

        # resident weights (contraction-major), loaded per 128-row chunk (Pool queue)
        woT_r = woT.rearrange("(t p) o -> p t o", p=128)
        # prefetch x tiles and rope tables for the first super-block before
        # the big weight loads so the first projection starts early
        xts0 = []
        for ct in range(8):
            xt_t = xt_p.tile([128, 512], bf16, tag="xt", name="xt0")
            nc.gpsimd.dma_start(xt_t[:], xT[ct * 128 : (ct + 1) * 128, 0:512])
            xts0.append(xt_t)
        tb_t0 = tb_p.tile([128, 2, 512], f32, tag="tb", name="tb0")
        nc.sync.dma_start(tb_t0[:], tabs[:, :, 0:512])
        wq_t = wres.tile([128, 8, DL], bf16, tag="wq")
        wk_t = wres.tile([128, 8, DL], bf16, tag="wk")
        wv_t = wres.tile([128, 8, DL], bf16, tag="wv")
        wo_t = wres.tile([128, 4, D], f32r, tag="wo")
        for ct in range(8):
            nc.sync.dma_start(wk_t[:, ct, :], wkT[ct * 128 : (ct + 1) * 128, :])
        for w_t, wT in ((wq_t, wqT), (wv_t, wvT)):
            for ct in range(8):
                nc.gpsimd.dma_start(w_t[:, ct, :], wT[ct * 128 : (ct + 1) * 128, :])
        # small constants after the weight loads (needed later than W)
        maskT_t = const.tile([128, 896], bf16, tag="maskT")
        nc.sync.dma_start(maskT_t[:], maskT[:])
        ident_t = const.tile([128, 128], bf16, tag="ident")
        nc.sync.dma_start(ident_t[:], ident[:])
        for it in range(4):
            nc.sync.dma_start(
                wo_t[:, it, :], woT_r[:, it, :].bitcast(f32r)
            )

        # persistent K / V buffers
        kt_tiles = {}
        for hp in range(4):
            for sbk in range(NSB):
                kt_tiles[hp, sbk] = kt_p.tile([128, 512], f32r, tag=f"kt{hp}_{sbk}", name=f"kt{hp}_{sbk}")
        v_tiles = {}
        for kb in range(NKB):
            v_tiles[kb] = vt_p.tile([128, HL, 65], f32r, tag=f"v{kb}", name=f"v{kb}")
            nc.vector.memset(v_tiles[kb][:, :, 64:65].bitcast(f32), 1.0)

        prev_out = [None]
        pending_norm = [None]
        next_x = [None]

        def load_x(s0n, name="xt"):
            xts_n = []
            for ct in range(8):
                xt_t = xt_p.tile([128, 512], bf16, tag="xt", name=name)
                eng = nc.gpsimd if ct % 2 == 0 else nc.sync
                eng.dma_start(xt_t[:], xT[ct * 128 : (ct + 1) * 128, s0n : s0n + 512])
                xts_n.append(xt_t)
            tb_n = tb_p.tile([128, 2, 512], f32, tag="tb", name="tb")
            nc.sync.dma_start(tb_n[:], tabs[:, :, s0n : s0n + 512])
            return xts_n, tb_n

        def emit_norm_a():
            # stage A of a head-pair's softmax normalization: evacuate the
            # denominator row to SBUF reshaped [128,4] (16B/partition DMA) so
            # the iterative-divide reciprocal costs 8 lane-elements, then
            # broadcast the reciprocals to 64 partitions on gpsimd. Emits no
            # DVE op that has to WAIT (the recip is stage B) so it never
            # blocks later DVE work queued behind it.
            hp_, cxs_, cxd_ = pending_norm[0]
            den_s = rc_p.tile([1, 2, 512], f32, tag="dens", name="dens")
            d4 = rc_p.tile([128, 2, 4], f32, tag="d4", name="d4")
            for par in (0, 1):
                nc.vector.tensor_copy(den_s[:, par, :], cxs_[par][64:65, :])
                nc.scalar.dma_start(d4[:, par, :], den_s[:, par, :])
            return d4

        def emit_norm_b(d4):
            r4 = rc_p.tile([128, 2, 4], f32, tag="r4", name="r4")
            rc0 = rc_p.tile([1, 512], f32, tag="rc0", name="rc0")
            rc1 = rc_p.tile([1, 512], f32, tag="rc1", name="rc1")
            rb_t = rc_p.tile([64, 2, 512], f32, tag="rb", name="rb")
            nc.vector.reciprocal(r4[:], d4[:])
            for par, rc_t in ((0, rc0), (1, rc1)):
                nc.gpsimd.dma_start(rc_t[:], r4[:, par, :])
                nc.gpsimd.partition_broadcast(rb_t[:, par, :], rc_t[:])
            return rb_t

        def emit_norm_c(rb_t):
            hp_, cxs_, cxd_ = pending_norm[0]
            pending_norm[0] = None
            cxt = cx_p.tile([128, 512], f32r, tag=f"cx{hp_}", name=f"cx{hp_}")
            cxd_[hp_] = cxt
            nc.vector.tensor_mul(cxt[0:64, :], cxs_[0][0:64, :], rb_t[:, 0, :])
            rl_t = rl_p.tile([64, 512], f32r, tag="rl", name="rl")
            nc.vector.tensor_mul(rl_t[:], cxs_[1][0:64, :], rb_t[:, 1, :])
            nc.sync.dma_start(cxt[64:128, :], rl_t[:])

        def emit_norm():
            if pending_norm[0] is None:
                return
            emit_norm_c(emit_norm_b(emit_norm_a()))

        for _rep in range(reps):
         for sb in range(NSB):
            s0 = sb * 512
            if _rep == 0 and sb == 0:
                xts, tb_t = xts0, tb_t0
            else:
                xts, tb_t = load_x(s0)
            # the previous super-block's last norm is flushed in stages woven
            # through this block's projection phase: each stage's ops only
            # execute once their inputs are surely ready, so they never block
            # the DVE/gpsimd queues that feed the rope -> projection pipeline
            norm_d4 = emit_norm_a() if pending_norm[0] is not None else None
            norm_rb = [None]

            # K and Q projections (transposed layout [f, s]) + RoPE
            qt_tiles = {}
            for w_t, is_k in ((wk_t, True), (wq_t, False)):
                for ft in range(4):
                    pr = pp.tile([128, 512], f32, tag="mm")
                    for ct in range(8):
                        nc.tensor.matmul(
                            pr[:],
                            w_t[:, ct, ft * 128 : (ft + 1) * 128],
                            xts[ct][:],
                            start=(ct == 0),
                            stop=(ct == 7),
                        )
                    # rope: dst = pr * cos + pairswap(pr) * (+-sin); both
                    # readers of pr (mul + shuffle) are issued first so the
                    # PSUM slot frees as fast as PE produces the next group
                    if is_k:
                        dst = kt_tiles[ft, sb]
                    else:
                        dst = qt_p.tile([128, 512], f32r, tag=f"qt{ft}", name=f"qt{ft}")
                        qt_tiles[ft] = dst
                    qs_t = qs_p.tile([128, 512], f32, tag="qs")
                    nc.vector.tensor_mul(dst[:], pr[:], tb_t[:, 0, :])
                    nc.vector.stream_shuffle(qs_t[:], pr[:], SWAP)
                    nc.vector.tensor_mul(qs_t[:], qs_t[:], tb_t[:, 1, :])
                    nc.vector.tensor_add(dst[:], dst[:], qs_t[:])
                if is_k and norm_d4 is not None:
                    norm_rb[0] = emit_norm_b(norm_d4)
            if norm_d4 is not None:
                emit_norm_c(norm_rb[0])

            # V projection (natural layout [s, f]) into the ones-augmented tiles
            for i in range(4):
                kb = sb * 4 + i
                pr = pp.tile([128, 512], f32, tag="mm")
                for ct in range(8):
                    nc.tensor.matmul(
                        pr[:],
                        xts[ct][:, i * 128 : (i + 1) * 128],
                        wv_t[:, ct, :],
                        start=(ct == 0),
                        stop=(ct == 7),
                    )
                nc.vector.tensor_copy(
                    v_tiles[kb][:, :, 0:64], pr[:].rearrange("p (h d) -> p h d", h=HL)
                )

            if prev_out[0] is not None:
                _emit_out_proj(nc, pp, os_p, wo_t, outp, *prev_out[0])
                prev_out[0] = None

            # attention for this query super-block, score->exp->V software-
            # pipelined. The V accumulation is ROTATED: blocks are summed in
            # order [2, 3, .., n-1, 0, 1], so each head-pair's first V matmul
            # (which must wait for the previous tenant of its PSUM bank to be
            # normalized) fires several score-iterations into the loop --
            # hiding the preceding norm chain's DVE latency from the PE.
            n_kj = 4 * (sb + 1)
            cx_tiles = {}
            norm_st = [None]  # staged mid-attention norm: [d4, rb]
            for hp in range(4):
                cxs = (
                    pc.tile([65, 512], f32, tag="ce", name="ce"),
                    pc.tile([65, 512], f32, tag="co", name="co"),
                )
                exts = {}
                spans = {}

                # rotation requires the start block (kj=2) to cover all 512
                # query columns, which holds only for sb >= 1; sb=0 keeps the
                # natural order (its V matmuls are all tail-emitted anyway)
                rot = n_kj > 4
                start_kj, stop_kj = (2, 1) if rot else (0, n_kj - 1)

                def emit_v(kj):
                    qo, w = spans[kj]
                    for par in (0, 1):
                        nc.tensor.matmul(
                            cxs[par][:, qo : qo + w],
                            v_tiles[kj][:, hp * 2 + par, :],
                            exts[kj][:, par, 0:w],
                            start=(kj == start_kj),
                            stop=(kj == stop_kj),
                        )
                    del exts[kj]

                for kj in range(n_kj):
                    diag = kj >= 4 * sb
                    kjl = kj - 4 * sb
                    w = max(512 - 128 * kjl, 256) if diag else 512
                    qo = 512 - w
                    moff = 384 - (128 * kjl - qo)
                    sbk, col = kj // 4, (kj % 4) * 128
                    spans[kj] = (qo, w)
                    scp = ps.tile([128, 2, 512], f32, tag="sc", name="sc")
                    # both 64-contraction score matmuls first, then (diag
                    # only) both 128-contraction mask matmuls: groups the PE
                    # row-tiling modes to halve array-drain mode switches
                    for par in (0, 1):
                        bp = 64 * par
                        kt_sl = kt_tiles[hp, sbk][bp : bp + 64, col : col + 128]
                        qt_sl = qt_tiles[hp][bp : bp + 64, qo : qo + w]
                        nc.tensor.matmul(
                            scp[:, par, qo : qo + w],
                            kt_sl,
                            qt_sl,
                            start=True,
                            stop=not diag,
                        )
                    if diag:
                        tw = 128 if kjl < 3 else 256
                        for par in (0, 1):
                            nc.tensor.matmul(
                                scp[:, par, qo : qo + tw],
                                ident_t[:],
                                maskT_t[:, moff : moff + tw],
                                start=False,
                                stop=True,
                            )
                    if rot and kj < 2:
                        # blocks 0/1 are V-accumulated last; their probs
                        # live in dedicated buffers until the tail
                        ext = ex_p.tile(
                            [128, 2, 512], f32r, tag=f"exh{kj}", name="exh", bufs=1
                        )
                    else:
                        ext = ex_p.tile([128, 2, 512], f32r, tag="ex", name="ex")
                    nc.scalar.activation(
                        ext[:, :, 0:w], scp[:, :, qo : qo + w], AF.Exp, scale=0.125
                    )
                    exts[kj] = ext
                    if norm_st[0] is not None:
                        if kj == 1:
                            norm_st[0][1] = emit_norm_b(norm_st[0][0])
                        elif kj == 3:
                            emit_norm_c(norm_st[0][1])
                            norm_st[0] = None
                    if rot:
                        # V blocks are emitted in batches of FOUR so the
                        # eight 128-contraction matmuls run back-to-back
                        # (one row-tiling mode switch instead of four)
                        if kj >= 7 and kj % 4 == 3:
                            for v in range(kj - 5, kj - 1):
                                emit_v(v)
                    elif kj >= 2:
                        emit_v(kj - 2)
                if rot:
                    vdone = 2 + 4 * max(0, (n_kj - 8) // 4 + 1)
                    for kj in range(max(2, vdone), n_kj):
                        emit_v(kj)
                    emit_v(0)
                    emit_v(1)
                else:
                    for kj in range(max(0, n_kj - 2), n_kj):
                        emit_v(kj)
                pending_norm[0] = (hp, cxs, cx_tiles)
                if hp < 3:
                    norm_st[0] = [emit_norm_a(), None]

            # defer: output projection of the PREVIOUS super-block is emitted
            # by the next iteration (after its projections) so its PSUM slots
            # don't gate the next projections; see emit_out_proj below.
            prev_out[0] = (s0, cx_tiles)
        emit_norm()
        if prev_out[0] is not None:
            _emit_out_proj(nc, pp, os_p, wo_t, outp, *prev_out[0])
            prev_out[0] = None


def _emit_out_proj(nc, os_psum, os_p, wo_t, outp, s0, cx_tiles):
    # Wo chunks are the STATIONARY operand ([128,128] -> 107ns LDWEIGHTS,
    # hidden under the 213ns matmul) and ctx the moving one; the output is
    # therefore feature-major [D, S] and the host transposes when gathering.
    from concourse import mybir

    f32 = mybir.dt.float32
    for og in range(2):
        ost = os_p.tile([128, 4, 512], f32, tag="os", name="ost")
        for i in range(4):
            oc = og * 4 + i
            opp = os_psum.tile([128, 512], f32, tag="mm", name="opp")
            for hp in range(4):
                nc.tensor.matmul(
                    opp[:],
                    wo_t[:, hp, oc * 128 : (oc + 1) * 128],
                    cx_tiles[hp][:],
                    start=(hp == 0),
                    stop=(hp == 3),
                )
            nc.vector.tensor_copy(ost[:, i, :], opp[:])
        eng = nc.sync if og == 0 else nc.scalar
        eng.dma_start(
            outp[og * 512 : (og + 1) * 512, s0 : s0 + 512].rearrange(
                "(t p) q -> p t q", p=128
            ),
            ost[:],
        )


def build(S=S_FULL, reps=1, chain=False):
    import concourse.tile as tile
    from concourse import bacc, mybir

    f32 = mybir.dt.float32
    bf16 = mybir.dt.bfloat16
    nc = bacc.Bacc(None, target_bir_lowering=False, debug=False)
    t = {}
    t["xT"] = nc.dram_tensor("xT", [D, S], bf16, kind="ExternalInput")
    t["wqT"] = nc.dram_tensor("wqT", [D, DL], bf16, kind="ExternalInput")
    t["wkT"] = nc.dram_tensor("wkT", [D, DL], bf16, kind="ExternalInput")
    t["wvT"] = nc.dram_tensor("wvT", [D, DL], bf16, kind="ExternalInput")
    t["woT"] = nc.dram_tensor("woT", [DL, D], f32, kind="ExternalInput")
    t["tabs"] = nc.dram_tensor("tabs", [128, 2, S], f32, kind="ExternalInput")
    t["maskT"] = nc.dram_tensor("maskT", [128, 896], mybir.dt.bfloat16, kind="ExternalInput")
    t["ident"] = nc.dram_tensor("ident", [128, 128], mybir.dt.bfloat16, kind="ExternalInput")
    t["outp"] = nc.dram_tensor("outp", [D, S], f32, kind="ExternalOutput")
    if chain:
        t["chain"] = nc.dram_tensor("chain", [128, 128], f32, kind="ExternalInput")
        t["chain_out"] = nc.dram_tensor("chain_out", [128, 128], f32, kind="ExternalOutput")

    with tile.TileContext(nc) as tc:
        _emit(nc, tc, t, S, reps=reps)
        if chain:
            with tc.tile_pool(name="chp", bufs=1) as chp:
                cht = chp.tile([128, 128], mybir.dt.float32, name="cht")
                nc.sync.dma_start(cht[:], t["chain"][:])
                nc.sync.dma_start(t["chain_out"][:], cht[:])
    nc.compile()
    return nc


def prep_inputs(x, Wq, Wk, Wv, Wo, token_positions, S=S_FULL):
    import ml_dtypes

    bf = ml_dtypes.bfloat16
    x = np.asarray(x)
    Wq, Wk, Wv, Wo = (np.asarray(a) for a in (Wq, Wk, Wv, Wo))
    pos = np.asarray(token_positions).astype(np.float64)
    inv = ROPE_THETA ** (-np.arange(0, DK, 2, dtype=np.float64) / DK)  # [32]
    ang = pos[:, None] * inv[None, :]  # [S, 32]
    cos = np.cos(ang).astype(np.float32).T  # [32, S]
    sin = np.sin(ang).astype(np.float32).T
    i_of_p = (np.arange(128) % 64) // 2
    c2 = cos[i_of_p, :]  # [128, S]
    sgn = np.where(np.arange(128) % 2 == 0, -1.0, 1.0).astype(np.float32)
    s2m = sin[i_of_p, :] * sgn[:, None]
    tabs = np.ascontiguousarray(np.stack([c2, s2m], axis=1))  # [128, 2, S]

    maskT = np.where(
        np.arange(896)[None, :] >= np.arange(128)[:, None] + 384, 0.0, NEG
    ).astype(ml_dtypes.bfloat16)
    ident = np.eye(128, dtype=ml_dtypes.bfloat16)

    nb = x.shape[0]
    maps = []
    for c in range(2 * nb):
        b, half = c // 2, c % 2
        rows = slice(half * DL, (half + 1) * DL)
        maps.append(
            {
                "xT": np.ascontiguousarray(x[b].T).astype(bf),
                "wqT": np.ascontiguousarray(Wq[rows].T).astype(bf),
                "wkT": np.ascontiguousarray(Wk[rows].T).astype(bf),
                "wvT": np.ascontiguousarray(Wv[rows].T).astype(bf),
                "woT": np.ascontiguousarray(Wo[:, rows].T),
                "tabs": tabs,
                "maskT": maskT,
                "ident": ident,
            }
        )
    return maps


def kernel(x, Wq, Wk, Wv, Wo, token_positions):
    from concourse.bass_utils import run_bass_kernel_spmd

    if "nc" not in _CACHE:
        _CACHE["nc"] = build()
    maps = prep_inputs(x, Wq, Wk, Wv, Wo, token_positions)
    res = run_bass_kernel_spmd(_CACHE["nc"], maps, list(range(8)))
    out = np.empty((B, S_FULL, D), np.float32)
    for b in range(B):
        out[b] = (res.results[2 * b]["outp"] + res.results[2 * b + 1]["outp"]).T
    return out

